# revision 4
# baseline (speedup 1.0000x reference)
"""Trainium2 Bass kernel for nn_ARSG (additive-attention style scoring with a
1-D conv over location features), data-parallel over batch across 8 NeuronCores.

Math (per batch b):
    f      = conv1d(F_matrix, a_prev[b])          # Toeplitz matmul over T
    x      = tanh(s_prev[b] @ Ww + hT[b] @ Vw + Vb + f @ Uw)
    e      = x @ ww
    out[b] = softmax(beta * e)

Key restructurings (validated vs the reference in fp64/fp32 mock):
  * Uw is folded into F on the host: G = F @ Uw, so Uf^T = G^T @ C_b^T where
    C_b^T is the (banded Toeplitz) conv coefficient matrix built from a_prev.
    This removes the separate f @ Uw matmul entirely (-25% FLOPs).
  * C_b^T tiles are materialized by DMA directly from a reversed, zero-padded
    copy of a_prev ("qrev") using an overlapping [1,128]x[1,512] access
    pattern.  Both matmul operands have their K-partitions reversed per
    128-block (G is block-reversed on the host), which keeps all AP steps
    positive while leaving the contraction sum unchanged.
  * h is transposed on the host to [b, DIM_H, T] so Vh^T accumulates into the
    same PSUM tile as Uf^T with K = DIM_H on partitions.
  * s_prev @ Ww + Vb (tiny) is computed on the host and applied as the
    per-partition bias of the tanh activation.
  * All matmuls run as float32r (full fp32 data; reduced-precision PE mode,
    1 cycle/row at N=512 -- same rate as bf16, ~4x faster than fp32).

Everything below T/B/... is hardcoded for the problem sizes:
    T=1024, B=32, DIM_F=512, DIM_H=512, DIM_S=1024, DIM_W=512, 8 cores.
"""

import numpy as np

T, B, DIM_F, DIM_H, DIM_S, DIM_W = 1024, 32, 512, 512, 1024, 512
N_CORES = 8
B_LOC = B // N_CORES  # batches per core
QLEN = 2048           # padded length of the reversed conv-coefficient vector

_program_cache: dict[float, object] = {}


def _build_program(beta: float):
    import concourse.bass as bass
    import concourse.mybir as mybir
    import concourse.tile as tile
    from concourse import bacc

    f32 = mybir.dt.float32
    f32r = mybir.dt.float32r
    bf16 = mybir.dt.bfloat16
    AFT = mybir.ActivationFunctionType

    nc = bacc.Bacc("TRN2", target_bir_lowering=False, debug=False)

    g_d = nc.dram_tensor("g", [T, DIM_W], bf16, kind="ExternalInput")
    vw_d = nc.dram_tensor("vw", [DIM_H, DIM_W], f32r, kind="ExternalInput")
    qr_d = nc.dram_tensor("qrev", [B_LOC, QLEN], bf16, kind="ExternalInput")
    ht_d = nc.dram_tensor("ht", [B_LOC, DIM_H, T], f32r, kind="ExternalInput")
    bias_d = nc.dram_tensor("bias", [128, B_LOC * 4], f32, kind="ExternalInput")
    wwr_d = nc.dram_tensor("wwr", [128, 4], f32r, kind="ExternalInput")
    out_d = nc.dram_tensor("out", [B_LOC, T], f32, kind="ExternalOutput")

    NKJ = T // 128       # 8 K-blocks for the conv contraction (over j)
    NKD = DIM_H // 128   # 4 K-blocks for the Vh contraction (over d)
    NWT = DIM_W // 128   # 4 output w-tiles
    NTC = T // 512       # 2 t-chunks of 512 (PSUM bank / fp32 moving-max)

    with tile.TileContext(nc) as tc:
        with (
            tc.tile_pool(name="const", bufs=1) as const_pool,
            tc.tile_pool(name="htp", bufs=2) as ht_pool,
            tc.tile_pool(name="convp", bufs=3) as conv_pool,
            tc.tile_pool(name="xp", bufs=2) as x_pool,
            tc.tile_pool(name="ep", bufs=2) as e_pool,
            tc.tile_pool(name="smallp", bufs=4) as small_pool,
            tc.tile_pool(name="psx", bufs=4, space="PSUM") as psx_pool,
            tc.tile_pool(name="pse", bufs=2, space="PSUM") as pse_pool,
        ):
            g_sb = const_pool.tile([128, NKJ, DIM_W], bf16)
            nc.sync.dma_start(
                out=g_sb[:],
                in_=bass.AP(tensor=g_d, offset=0,
                            ap=[[DIM_W, 128], [128 * DIM_W, NKJ], [1, DIM_W]]),
            )
            vw_sb = const_pool.tile([128, NKD, DIM_W], f32r)
            nc.sync.dma_start(
                out=vw_sb[:],
                in_=bass.AP(tensor=vw_d, offset=0,
                            ap=[[DIM_W, 128], [128 * DIM_W, NKD], [1, DIM_W]]),
            )
            bias_sb = const_pool.tile([128, B_LOC * 4], f32)
            nc.sync.dma_start(out=bias_sb[:], in_=bias_d.ap()[:])
            wwr_sb = const_pool.tile([128, 4], f32r)
            nc.sync.dma_start(out=wwr_sb[:], in_=wwr_d.ap()[:])

            for lb in range(B_LOC):
                ht_sb = ht_pool.tile([128, NKD, T], f32r, tag="ht")
                nc.sync.dma_start(
                    out=ht_sb[:],
                    in_=bass.AP(tensor=ht_d, offset=lb * DIM_H * T,
                                ap=[[T, 128], [128 * T, NKD], [1, T]]),
                )
                e_sb = e_pool.tile([1, T], f32, tag="e")
                for tch in range(NTC):
                    # Conv coefficient tiles: C_b^T[kj-block (rev), t-chunk],
                    # read as overlapping windows of qrev.  One 3D DMA loads
                    # all 8 kj-blocks; the kj axis is stored reversed
                    # (slot kjp holds block kj = NKJ-1-kjp) so every AP step
                    # stays positive.
                    ct8 = conv_pool.tile([128, NKJ, 512], bf16, tag="conv",
                                         name=f"ct8_{lb}_{tch}")
                    nc.sync.dma_start(
                        out=ct8[:],
                        in_=bass.AP(tensor=qr_d,
                                    offset=lb * QLEN + 512 * tch,
                                    ap=[[1, 128], [128, NKJ], [1, 512]]),
                    )
                    if tch == 0:
                        # t=0 output row of the conv is zero by construction
                        # (even-T padding in the reference); overwrite the
                        # junk column-0 coefficients (kj<4 <=> kjp>=4) with a
                        # known-zero qrev element.
                        zsrc = bass.AP(tensor=qr_d, offset=QLEN - 1,
                                       ap=[[0, 128], [0, 4], [0, 1]])
                        nc.sync.dma_start(out=ct8[:, 4:NKJ, 0:1], in_=zsrc)
                    x_sb = x_pool.tile([128, NWT, 512], f32r, tag="x")
                    for wt in range(NWT):
                        ps = psx_pool.tile([128, 512], f32, tag="psx",
                                           name=f"ps_{lb}_{tch}_{wt}")
                        for kj in range(NKJ):
                            nc.tensor.matmul(
                                ps[:],
                                g_sb[:, kj, wt * 128:(wt + 1) * 128],
                                ct8[:, NKJ - 1 - kj, :],
                                start=(kj == 0), stop=False,
                            )
                        for kd in range(NKD):
                            nc.tensor.matmul(
                                ps[:],
                                vw_sb[:, kd, wt * 128:(wt + 1) * 128],
                                ht_sb[:, kd, tch * 512:(tch + 1) * 512],
                                start=False, stop=(kd == NKD - 1),
                            )
                        nc.scalar.activation(
                            x_sb[:, wt, :], ps[:], AFT.Tanh,
                            bias=bias_sb[:, lb * 4 + wt: lb * 4 + wt + 1],
                        )
                    pe = pse_pool.tile([1, 512], f32, tag="pse",
                                       name=f"pe_{lb}_{tch}")
                    for wt in range(NWT):
                        nc.tensor.matmul(
                            pe[:],
                            wwr_sb[:, wt:wt + 1],
                            x_sb[:, wt, :],
                            start=(wt == 0), stop=(wt == NWT - 1),
                        )
                    nc.vector.tensor_copy(e_sb[:, tch * 512:(tch + 1) * 512], pe[:])

                # softmax(beta * e) over the full row (single partition)
                mx = small_pool.tile([1, 1], f32, tag="mx", name=f"mx_{lb}")
                nc.vector.reduce_max(mx[:], e_sb[:], axis=mybir.AxisListType.X)
                nbm = small_pool.tile([1, 1], f32, tag="nbm", name=f"nbm_{lb}")
                nc.vector.tensor_scalar_mul(nbm[:], mx[:], -float(beta))
                p_sb = e_pool.tile([1, T], f32, tag="p", name=f"p_{lb}")
                ssum = small_pool.tile([1, 1], f32, tag="ssum", name=f"ssum_{lb}")
                nc.scalar.activation(
                    p_sb[:], e_sb[:], AFT.Exp,
                    bias=nbm[:], scale=float(beta), accum_out=ssum[:],
                )
                rec = small_pool.tile([1, 1], f32, tag="rec", name=f"rec_{lb}")
                nc.vector.reciprocal(rec[:], ssum[:])
                o_sb = e_pool.tile([1, T], f32, tag="o", name=f"o_{lb}")
                nc.vector.tensor_scalar_mul(o_sb[:], p_sb[:], rec[:])
                nc.sync.dma_start(out=out_d.ap()[lb:lb + 1, :], in_=o_sb[:])

    nc.compile()
    return nc


def _get_program(beta: float):
    if beta not in _program_cache:
        _program_cache[beta] = _build_program(beta)
    return _program_cache[beta]


def _prepare_in_maps(F, a_prev, s_prev, h, Ww, Vw, Vb, Uw, ww):
    """Host-side sharding + layout prep. Cheap (one small matmul + copies)."""
    import ml_dtypes
    bf16 = ml_dtypes.bfloat16
    G = (F.astype(np.float64) @ Uw.astype(np.float64)).astype(np.float32)
    # Reverse each 128-row block of G so conv lhsT/rhs partition orders match.
    G_br = np.ascontiguousarray(
        G.reshape(T // 128, 128, DIM_W)[:, ::-1, :].reshape(T, DIM_W)
    ).astype(bf16)
    Ws = (s_prev.astype(np.float64) @ Ww.astype(np.float64)).astype(np.float32)
    Ws = Ws + Vb[None, :]                                   # [B, DIM_W]
    wwr = np.ascontiguousarray(ww.reshape(4, 128).T)        # [128, 4]

    in_maps = []
    for core in range(N_CORES):
        b0 = core * B_LOC
        ppad = np.zeros((B_LOC, 2 * T - 1), np.float32)
        ppad[:, T // 2 - 1: T // 2 - 1 + T] = a_prev[b0:b0 + B_LOC]
        qrev = np.zeros((B_LOC, QLEN), np.float32)
        qrev[:, : 2 * T - 1] = ppad[:, ::-1]
        qrev = qrev.astype(bf16)
        hT = np.ascontiguousarray(h[:, b0:b0 + B_LOC, :].transpose(1, 2, 0))
        bias_core = np.ascontiguousarray(
            Ws[b0:b0 + B_LOC].reshape(B_LOC, 4, 128).transpose(2, 0, 1)
            .reshape(128, B_LOC * 4)
        )
        in_maps.append({
            "g": G_br, "vw": np.ascontiguousarray(Vw), "qrev": qrev,
            "ht": hT, "bias": bias_core, "wwr": wwr,
        })
    return in_maps


def kernel(**inputs: np.ndarray) -> np.ndarray:
    F = np.ascontiguousarray(np.asarray(inputs["F_matrix"], dtype=np.float32))
    a_prev = np.ascontiguousarray(np.asarray(inputs["a_prev"], dtype=np.float32))
    s_prev = np.ascontiguousarray(np.asarray(inputs["s_prev"], dtype=np.float32))
    h = np.ascontiguousarray(np.asarray(inputs["h"], dtype=np.float32))
    Ww = np.asarray(inputs["Ww"], dtype=np.float32)
    Vw = np.asarray(inputs["Vw"], dtype=np.float32)
    Vb = np.asarray(inputs["Vb"], dtype=np.float32)
    Uw = np.asarray(inputs["Uw"], dtype=np.float32)
    ww = np.asarray(inputs["ww"], dtype=np.float32)
    beta = float(np.asarray(inputs["beta"]))

    nc = _get_program(beta)
    in_maps = _prepare_in_maps(F, a_prev, s_prev, h, Ww, Vw, Vb, Uw, ww)

    from concourse.bass_utils import run_bass_kernel_spmd

    res = run_bass_kernel_spmd(nc, in_maps, core_ids=list(range(N_CORES)))
    out = np.concatenate(
        [res.results[i]["out"] for i in range(N_CORES)], axis=0
    ).astype(np.float32)
    return out


# revision 6
# speedup vs baseline: 1.3301x; 1.3301x over previous
"""Trainium2 Bass kernel for nn_ARSG (additive-attention style scoring with a
1-D conv over location features), data-parallel over batch across 8 NeuronCores.

Math (per batch b):
    f      = conv1d(F_matrix, a_prev[b])          # Toeplitz matmul over T
    x      = tanh(s_prev[b] @ Ww + hT[b] @ Vw + Vb + f @ Uw)
    e      = x @ ww
    out[b] = softmax(beta * e)

Key restructurings (validated vs the reference in fp64/fp32 mock):
  * Uw is folded into F on the host: G = F @ Uw, so Uf^T = G^T @ C_b^T where
    C_b^T is the (banded Toeplitz) conv coefficient matrix built from a_prev.
    This removes the separate f @ Uw matmul entirely (-25% FLOPs).
  * C_b^T tiles are materialized by DMA directly from a reversed, zero-padded
    copy of a_prev ("qrev") using an overlapping [1,128]x[1,512] access
    pattern.  Both matmul operands have their K-partitions reversed per
    128-block (G is block-reversed on the host), which keeps all AP steps
    positive while leaving the contraction sum unchanged.
  * h is transposed on the host to [b, DIM_H, T] so Vh^T accumulates into the
    same PSUM tile as Uf^T with K = DIM_H on partitions.
  * s_prev @ Ww + Vb (tiny) is computed on the host and applied as the
    per-partition bias of the tanh activation.
  * All matmuls run as float32r (full fp32 data; reduced-precision PE mode,
    1 cycle/row at N=512 -- same rate as bf16, ~4x faster than fp32).

Everything below T/B/... is hardcoded for the problem sizes:
    T=1024, B=32, DIM_F=512, DIM_H=512, DIM_S=1024, DIM_W=512, 8 cores.
"""

import numpy as np

T, B, DIM_F, DIM_H, DIM_S, DIM_W = 1024, 32, 512, 512, 1024, 512
N_CORES = 8
B_LOC = B // N_CORES  # batches per core
QLEN = 2048           # padded length of the reversed conv-coefficient vector

_program_cache: dict[float, object] = {}


def _build_program(beta: float):
    import concourse.bass as bass
    import concourse.mybir as mybir
    import concourse.tile as tile
    from concourse import bacc

    f32 = mybir.dt.float32
    f32r = mybir.dt.float32r
    bf16 = mybir.dt.bfloat16
    AFT = mybir.ActivationFunctionType

    nc = bacc.Bacc("TRN2", target_bir_lowering=False, debug=False)

    g_d = nc.dram_tensor("g", [T, DIM_W], bf16, kind="ExternalInput")
    vw_d = nc.dram_tensor("vw", [DIM_H, DIM_W], f32r, kind="ExternalInput")
    qr_d = nc.dram_tensor("qrev", [B_LOC, QLEN], bf16, kind="ExternalInput")
    ht_d = nc.dram_tensor("ht", [B_LOC, DIM_H, T], f32r, kind="ExternalInput")
    bias_d = nc.dram_tensor("bias", [128, B_LOC * 4], f32, kind="ExternalInput")
    wwr_d = nc.dram_tensor("wwr", [128, 4], f32r, kind="ExternalInput")
    out_d = nc.dram_tensor("out", [B_LOC, T], f32, kind="ExternalOutput")

    NKJ = T // 128       # 8 K-blocks for the conv contraction (over j)
    NKD = DIM_H // 128   # 4 K-blocks for the Vh contraction (over d)
    NWT = DIM_W // 128   # 4 output w-tiles
    NTC = T // 512       # 2 t-chunks of 512 (PSUM bank / fp32 moving-max)

    with tile.TileContext(nc) as tc:
        with (
            tc.tile_pool(name="const", bufs=1) as const_pool,
            tc.tile_pool(name="htp", bufs=2) as ht_pool,
            tc.tile_pool(name="convp", bufs=3) as conv_pool,
            tc.tile_pool(name="xp", bufs=2) as x_pool,
            tc.tile_pool(name="ep", bufs=2) as e_pool,
            tc.tile_pool(name="smallp", bufs=4) as small_pool,
            tc.tile_pool(name="psx", bufs=4, space="PSUM") as psx_pool,
            tc.tile_pool(name="pse", bufs=2, space="PSUM") as pse_pool,
        ):
            g_sb = const_pool.tile([128, NKJ, DIM_W], bf16)
            nc.sync.dma_start(
                out=g_sb[:],
                in_=bass.AP(tensor=g_d, offset=0,
                            ap=[[DIM_W, 128], [128 * DIM_W, NKJ], [1, DIM_W]]),
            )
            vw_sb = const_pool.tile([128, NKD, DIM_W], f32r)
            nc.sync.dma_start(
                out=vw_sb[:],
                in_=bass.AP(tensor=vw_d, offset=0,
                            ap=[[DIM_W, 128], [128 * DIM_W, NKD], [1, DIM_W]]),
            )
            bias_sb = const_pool.tile([128, B_LOC * 4], f32)
            nc.sync.dma_start(out=bias_sb[:], in_=bias_d.ap()[:])
            wwr_sb = const_pool.tile([128, 4], f32r)
            nc.sync.dma_start(out=wwr_sb[:], in_=wwr_d.ap()[:])

            for lb in range(B_LOC):
                ht_sb = ht_pool.tile([128, NKD, T], f32r, tag="ht")
                nc.sync.dma_start(
                    out=ht_sb[:],
                    in_=bass.AP(tensor=ht_d, offset=lb * DIM_H * T,
                                ap=[[T, 128], [128 * T, NKD], [1, T]]),
                )
                # All conv coefficient tiles for this batch are overlapping
                # windows of qrev, and they all live inside ONE [128, 1920]
                # window: W[p, c] = qrev[lb, p + c].  The rhs for (kj, tch) is
                # the slice W[:, 512*tch + 128*(NKJ-1-kj) :+ 512].
                w_sb = conv_pool.tile([128, 1920], bf16, tag="conv",
                                      name=f"w_{lb}")
                nc.sync.dma_start(
                    out=w_sb[:],
                    in_=bass.AP(tensor=qr_d, offset=lb * QLEN,
                                ap=[[1, 128], [1, 1920]]),
                )
                e_sb = e_pool.tile([1, T], f32, tag="e")
                for tch in range(NTC):
                    x_sb = x_pool.tile([128, NWT, 512], f32r, tag="x")
                    for wt in range(NWT):
                        ps = psx_pool.tile([128, 512], f32, tag="psx",
                                           name=f"ps_{lb}_{tch}_{wt}")
                        # kj >= 4 first: at tch==0 their t=0 coefficients are
                        # naturally zero, so the start=True matmul covers the
                        # full 512 columns; the kj < 4 blocks carry junk t=0
                        # coefficients (the t=0 conv output row is zero by
                        # construction -- even-T padding in the reference), so
                        # they skip column 0 (N=511) and just accumulate.
                        kj_order = list(range(4, NKJ)) + list(range(4))
                        for mi, kj in enumerate(kj_order):
                            c0 = 512 * tch + 128 * (NKJ - 1 - kj)
                            if tch == 0 and kj < 4:
                                nc.tensor.matmul(
                                    ps[:, 1:],
                                    g_sb[:, kj, wt * 128:(wt + 1) * 128],
                                    w_sb[:, c0 + 1: c0 + 512],
                                    start=False, stop=False,
                                )
                            else:
                                nc.tensor.matmul(
                                    ps[:],
                                    g_sb[:, kj, wt * 128:(wt + 1) * 128],
                                    w_sb[:, c0: c0 + 512],
                                    start=(mi == 0), stop=False,
                                )
                        for kd in range(NKD):
                            nc.tensor.matmul(
                                ps[:],
                                vw_sb[:, kd, wt * 128:(wt + 1) * 128],
                                ht_sb[:, kd, tch * 512:(tch + 1) * 512],
                                start=False, stop=(kd == NKD - 1),
                            )
                        nc.scalar.activation(
                            x_sb[:, wt, :], ps[:], AFT.Tanh,
                            bias=bias_sb[:, lb * 4 + wt: lb * 4 + wt + 1],
                        )
                    pe = pse_pool.tile([1, 512], f32, tag="pse",
                                       name=f"pe_{lb}_{tch}")
                    for wt in range(NWT):
                        nc.tensor.matmul(
                            pe[:],
                            wwr_sb[:, wt:wt + 1],
                            x_sb[:, wt, :],
                            start=(wt == 0), stop=(wt == NWT - 1),
                        )
                    nc.vector.tensor_copy(e_sb[:, tch * 512:(tch + 1) * 512], pe[:])

                # softmax(beta * e) over the full row (single partition)
                mx = small_pool.tile([1, 1], f32, tag="mx", name=f"mx_{lb}")
                nc.vector.reduce_max(mx[:], e_sb[:], axis=mybir.AxisListType.X)
                nbm = small_pool.tile([1, 1], f32, tag="nbm", name=f"nbm_{lb}")
                nc.vector.tensor_scalar_mul(nbm[:], mx[:], -float(beta))
                p_sb = e_pool.tile([1, T], f32, tag="p", name=f"p_{lb}")
                ssum = small_pool.tile([1, 1], f32, tag="ssum", name=f"ssum_{lb}")
                nc.scalar.activation(
                    p_sb[:], e_sb[:], AFT.Exp,
                    bias=nbm[:], scale=float(beta), accum_out=ssum[:],
                )
                rec = small_pool.tile([1, 1], f32, tag="rec", name=f"rec_{lb}")
                nc.vector.reciprocal(rec[:], ssum[:])
                o_sb = e_pool.tile([1, T], f32, tag="o", name=f"o_{lb}")
                nc.vector.tensor_scalar_mul(o_sb[:], p_sb[:], rec[:])
                nc.sync.dma_start(out=out_d.ap()[lb:lb + 1, :], in_=o_sb[:])

    nc.compile()
    return nc


def _get_program(beta: float):
    if beta not in _program_cache:
        _program_cache[beta] = _build_program(beta)
    return _program_cache[beta]


def _prepare_in_maps(F, a_prev, s_prev, h, Ww, Vw, Vb, Uw, ww):
    """Host-side sharding + layout prep. Cheap (one small matmul + copies)."""
    import ml_dtypes
    bf16 = ml_dtypes.bfloat16
    G = (F.astype(np.float64) @ Uw.astype(np.float64)).astype(np.float32)
    # Reverse each 128-row block of G so conv lhsT/rhs partition orders match.
    G_br = np.ascontiguousarray(
        G.reshape(T // 128, 128, DIM_W)[:, ::-1, :].reshape(T, DIM_W)
    ).astype(bf16)
    Ws = (s_prev.astype(np.float64) @ Ww.astype(np.float64)).astype(np.float32)
    Ws = Ws + Vb[None, :]                                   # [B, DIM_W]
    wwr = np.ascontiguousarray(ww.reshape(4, 128).T)        # [128, 4]

    in_maps = []
    for core in range(N_CORES):
        b0 = core * B_LOC
        ppad = np.zeros((B_LOC, 2 * T - 1), np.float32)
        ppad[:, T // 2 - 1: T // 2 - 1 + T] = a_prev[b0:b0 + B_LOC]
        qrev = np.zeros((B_LOC, QLEN), np.float32)
        qrev[:, : 2 * T - 1] = ppad[:, ::-1]
        qrev = qrev.astype(bf16)
        hT = np.ascontiguousarray(h[:, b0:b0 + B_LOC, :].transpose(1, 2, 0))
        bias_core = np.ascontiguousarray(
            Ws[b0:b0 + B_LOC].reshape(B_LOC, 4, 128).transpose(2, 0, 1)
            .reshape(128, B_LOC * 4)
        )
        in_maps.append({
            "g": G_br, "vw": np.ascontiguousarray(Vw), "qrev": qrev,
            "ht": hT, "bias": bias_core, "wwr": wwr,
        })
    return in_maps


def kernel(**inputs: np.ndarray) -> np.ndarray:
    F = np.ascontiguousarray(np.asarray(inputs["F_matrix"], dtype=np.float32))
    a_prev = np.ascontiguousarray(np.asarray(inputs["a_prev"], dtype=np.float32))
    s_prev = np.ascontiguousarray(np.asarray(inputs["s_prev"], dtype=np.float32))
    h = np.ascontiguousarray(np.asarray(inputs["h"], dtype=np.float32))
    Ww = np.asarray(inputs["Ww"], dtype=np.float32)
    Vw = np.asarray(inputs["Vw"], dtype=np.float32)
    Vb = np.asarray(inputs["Vb"], dtype=np.float32)
    Uw = np.asarray(inputs["Uw"], dtype=np.float32)
    ww = np.asarray(inputs["ww"], dtype=np.float32)
    beta = float(np.asarray(inputs["beta"]))

    nc = _get_program(beta)
    in_maps = _prepare_in_maps(F, a_prev, s_prev, h, Ww, Vw, Vb, Uw, ww)

    from concourse.bass_utils import run_bass_kernel_spmd

    res = run_bass_kernel_spmd(nc, in_maps, core_ids=list(range(N_CORES)))
    out = np.concatenate(
        [res.results[i]["out"] for i in range(N_CORES)], axis=0
    ).astype(np.float32)
    return out
